# revision 1
# baseline (speedup 1.0000x reference)
"""Dictionary-learning matching-pursuit kernel for TRN2 (8 NeuronCores).

Algorithm (per sample x):
    proj = x @ D                      # [atoms]
    repeat sparsity times:
        best = argmax |proj|          # abs-argmax, first index on ties
        coef = proj[best]
        recon += coef * D[:, best]
        proj -= coef * G[best, :]     # G = D^T D  (Gram recurrence)

Sharding: data-parallel over the batch across 8 cores; the dictionary +
Gram matrix are replicated (computed redundantly per core).

Device layout per core (1024 samples):
  - proj kept resident in SBUF as 8 tiles of [128, 4096] f32.
  - W = [G | D^T] in core-local DRAM ([4096, 4608] f32) so one indirect
    DMA per tile-step gathers both the Gram row and the dictionary column.
  - Per step+tile: max_index finds the +/-absmax locations (sign and
    first-index tie resolution via unsigned min over the two candidate
    indices), indirect-DMA row gather, ACT scales the row by coef
    in-place, one fused tensor_tensor_reduce subtracts the scaled Gram
    row from proj while producing the next step's absmax.
"""

import numpy as np

import concourse.bacc as bacc
import concourse.mybir as mybir
from concourse.bass import IndirectOffsetOnAxis
from concourse.bass_utils import run_bass_kernel_spmd
from concourse.masks import make_identity
from concourse.tile import TileContext



P = 128
FEAT = 512
ATOMS = 4096
BATCH = 8192
NCORES = 8
F32 = mybir.dt.float32
U32 = mybir.dt.uint32
# how many sample-tiles run their proj update on GpSimd instead of DVE
_GP_SUB_TILES = 0


def emit_pursuit(tc, X, D, OUT, W, *, b_sh, feat, atoms, sparsity):
    """Emit the full per-core program into TileContext tc.

    X:   [b_sh, feat] f32 DRAM input (this core's batch shard)
    D:   [feat, atoms] f32 DRAM input (replicated dictionary)
    OUT: [b_sh, feat] f32 DRAM output (reconstruction)
    W:   [atoms, atoms + feat] f32 DRAM scratch ([G | D^T])
    """
    nc = tc.nc
    KC = feat // P        # contraction chunks for matmuls
    NB = atoms // 512     # 512-wide atom blocks
    MB = atoms // P       # 128-row atom blocks
    ST = b_sh // P        # sample tiles
    WIDE = atoms + feat

    with (
        tc.tile_pool(name="const", bufs=1) as constp,
        tc.tile_pool(name="persist", bufs=1) as persist,
        tc.tile_pool(name="psum", bufs=2, space="PSUM") as psum,
    ):
        ident = constp.tile([P, P], F32, tag="ident")
        make_identity(nc, ident[:])

        # proj tiles stay resident in SBUF for the whole kernel
        Pt = [persist.tile([P, atoms], F32, tag=f"proj{si}", name=f"proj{si}") for si in range(ST)]
        # per-tile absmax |v| — persists across steps
        Av = [persist.tile([P, 1], F32, tag=f"absv{si}", name=f"absv{si}") for si in range(ST)]

        # ---------- Phase 1: W = [G | D^T] ----------
        with (
            tc.tile_pool(name="dsb", bufs=1) as dsbp,
            tc.tile_pool(name="gst", bufs=3) as gst,
        ):
            D_sb = dsbp.tile([P, KC * atoms], F32, tag="dsb")
            for c in range(KC):
                nc.sync.dma_start(
                    out=D_sb[:, c * atoms:(c + 1) * atoms],
                    in_=D[c * P:(c + 1) * P, :],
                )
            # G is symmetric: compute only blocks on/right of the diagonal
            # quad (nj >= mi//4); fill the strict lower triangle with PE
            # transposes of the staged upper blocks.
            for mi in range(MB):
                for nj in range(mi // 4, NB):
                    ps = psum.tile([P, 512], F32, tag="mmps")
                    for c in range(KC):
                        nc.tensor.matmul(
                            ps[:],
                            lhsT=D_sb[:, c * atoms + mi * P:c * atoms + mi * P + P],
                            rhs=D_sb[:, c * atoms + nj * 512:c * atoms + nj * 512 + 512],
                            start=(c == 0),
                            stop=(c == KC - 1),
                        )
                    st = gst.tile([P, 512], F32, tag="gstage")
                    nc.scalar.copy(st[:], ps[:])
                    nc.sync.dma_start(
                        out=W[mi * P:(mi + 1) * P, nj * 512:(nj + 1) * 512],
                        in_=st[:],
                    )
                    if nj > mi // 4:
                        for c in range(4):
                            pst = psum.tile([P, P], F32, tag="trps")
                            nc.tensor.transpose(
                                pst[:], st[:, c * P:(c + 1) * P], ident[:]
                            )
                            st2 = gst.tile([P, P], F32, tag="tstage")
                            nc.vector.tensor_copy(st2[:], pst[:])
                            nc.sync.dma_start(
                                out=W[(4 * nj + c) * P:(4 * nj + c + 1) * P,
                                      mi * P:(mi + 1) * P],
                                in_=st2[:],
                            )
            # D^T into the last `feat` columns of W
            for mi in range(MB):
                for c in range(KC):
                    pst = psum.tile([P, P], F32, tag="trps")
                    nc.tensor.transpose(
                        pst[:],
                        D_sb[:, c * atoms + mi * P:c * atoms + mi * P + P],
                        ident[:],
                    )
                    st2 = gst.tile([P, P], F32, tag="tstage")
                    nc.vector.tensor_copy(st2[:], pst[:])
                    nc.sync.dma_start(
                        out=W[mi * P:(mi + 1) * P, atoms + c * P:atoms + (c + 1) * P],
                        in_=st2[:],
                    )

        # ---------- Phase 2: proj0 = X @ D ----------
        with (
            tc.tile_pool(name="xt", bufs=1) as xtp,
            tc.tile_pool(name="xload", bufs=2) as xload,
            tc.tile_pool(name="dstream", bufs=2) as dstream,
        ):
            XT = xtp.tile([P, KC * b_sh], F32, tag="xtsb")
            for si in range(ST):
                xl = xload.tile([P, feat], F32, tag="xl")
                nc.sync.dma_start(out=xl[:], in_=X[si * P:(si + 1) * P, :])
                for c in range(KC):
                    pst = psum.tile([P, P], F32, tag="trps")
                    nc.tensor.transpose(pst[:], xl[:, c * P:(c + 1) * P], ident[:])
                    nc.vector.tensor_copy(
                        XT[:, c * b_sh + si * P:c * b_sh + si * P + P], pst[:]
                    )
            for nj in range(NB):
                dnj = dstream.tile([P, KC * 512], F32, tag="dnj")
                for c in range(KC):
                    nc.sync.dma_start(
                        out=dnj[:, c * 512:(c + 1) * 512],
                        in_=D[c * P:(c + 1) * P, nj * 512:(nj + 1) * 512],
                    )
                for si in range(ST):
                    ps = psum.tile([P, 512], F32, tag="mmps")
                    for c in range(KC):
                        nc.tensor.matmul(
                            ps[:],
                            lhsT=XT[:, c * b_sh + si * P:c * b_sh + si * P + P],
                            rhs=dnj[:, c * 512:(c + 1) * 512],
                            start=(c == 0),
                            stop=(c == KC - 1),
                        )
                    nc.scalar.copy(Pt[si][:, nj * 512:(nj + 1) * 512], ps[:])

        # W writes must land before the loop's gathers
        tc.strict_bb_all_engine_barrier()

        # ---------- Phase 3: pursuit loop ----------
        with (
            tc.tile_pool(name="wrow", bufs=3) as wpool,
            tc.tile_pool(name="smallf", bufs=4) as smallf,
            tc.tile_pool(name="smalli", bufs=4) as smalli,
            tc.tile_pool(name="reconp", bufs=1) as reconp,
        ):
            Rt = [reconp.tile([P, feat], F32, tag=f"recon{si}", name=f"recon{si}") for si in range(ST)]
            for si in range(ST):
                nc.vector.memset(Rt[si][:], 0.0)
                # absmax |v| of the initial projections
                nc.vector.tensor_reduce(
                    out=Av[si][:], in_=Pt[si][:],
                    axis=mybir.AxisListType.X, op=mybir.AluOpType.max,
                    apply_absolute_value=True,
                )

            # GP_SUB tiles get their proj update on GpSimd, the rest on DVE
            gp_sub = int(_GP_SUB_TILES) if "_GP_SUB_TILES" in globals() else 0
            for t in range(sparsity):
                wrows = []
                # phase A: per tile, find the atom and launch its row gather
                for si in range(ST):
                    # search for +|v| (slots 0-3) and -|v| (slots 4-7)
                    negv = smallf.tile([P, 1], F32, tag="negv", name="negv")
                    nc.scalar.mul(negv[:], Av[si][:], -1.0)
                    vpm = smallf.tile([P, 8], F32, tag="vpm", name="vpm")
                    nc.scalar.copy(vpm[:, 0:4], Av[si][:, 0:1].to_broadcast([P, 4]))
                    nc.scalar.copy(vpm[:, 4:8], negv[:, 0:1].to_broadcast([P, 4]))
                    idx8 = smalli.tile([P, 8], U32, tag="idx8", name="idx8")
                    nc.vector.max_index(idx8[:], vpm[:], Pt[si][:])
                    # unmatched slots read 0xFFFFFFFF (verified on HW), so
                    # unsigned min picks the real hit; +v/-v double-hit picks
                    # the earlier index (np.argmax first-occurrence tie rule).
                    idxm = smalli.tile([P, 1], U32, tag="idxm", name="idxm")
                    nc.vector.tensor_tensor(
                        out=idxm[:], in0=idx8[:, 0:1], in1=idx8[:, 4:5],
                        op=mybir.AluOpType.min,
                    )
                    msk = smalli.tile([P, 1], U32, tag="msk", name="msk")
                    nc.vector.tensor_tensor(
                        out=msk[:], in0=idx8[:, 0:1], in1=idx8[:, 4:5],
                        op=mybir.AluOpType.is_lt,
                    )
                    coef = smallf.tile([P, 1], F32, tag="coef", name="coef")
                    nc.vector.select(coef[:], msk[:], Av[si][:], negv[:])

                    wrow = wpool.tile([P, WIDE], F32, tag="wrow", name="wrow")
                    nc.gpsimd.indirect_dma_start(
                        out=wrow[:],
                        out_offset=None,
                        in_=W[:, :],
                        in_offset=IndirectOffsetOnAxis(ap=idxm[:, 0:1], axis=0),
                    )
                    # scale the whole gathered row by coef (in place, ACT)
                    nc.scalar.mul(wrow[:], wrow[:], coef[:, 0:1])
                    wrows.append(wrow)
                # phase B: apply the updates
                for si in range(ST):
                    wrow = wrows[si]
                    # recon += coef * D[:, best] (DVE, small)
                    nc.vector.tensor_add(Rt[si][:], Rt[si][:], wrow[:, atoms:])
                    if t < sparsity - 1:
                        # proj -= coef * G[best], then next step's absmax
                        sub_eng = nc.gpsimd if si < gp_sub else nc.vector
                        sub_eng.tensor_tensor(
                            out=Pt[si][:], in0=Pt[si][:], in1=wrow[:, :atoms],
                            op=mybir.AluOpType.subtract,
                        )
                        nc.vector.tensor_reduce(
                            out=Av[si][:], in_=Pt[si][:],
                            axis=mybir.AxisListType.X, op=mybir.AluOpType.max,
                            apply_absolute_value=True,
                        )

            for si in range(ST):
                nc.sync.dma_start(out=OUT[si * P:(si + 1) * P, :], in_=Rt[si][:])


def build_program(sparsity, b_sh=BATCH // NCORES, feat=FEAT, atoms=ATOMS):
    nc = bacc.Bacc("TRN2", target_bir_lowering=False, debug=False)
    X = nc.dram_tensor("X", [b_sh, feat], F32, kind="ExternalInput")
    D = nc.dram_tensor("dictionary", [feat, atoms], F32, kind="ExternalInput")
    OUT = nc.dram_tensor("recon", [b_sh, feat], F32, kind="ExternalOutput")
    W = nc.dram_tensor("W", [atoms, atoms + feat], F32, kind="Internal")
    with TileContext(nc) as tc:
        emit_pursuit(
            tc, X.ap(), D.ap(), OUT.ap(), W.ap(),
            b_sh=b_sh, feat=feat, atoms=atoms, sparsity=sparsity,
        )
    nc.compile()
    return nc


def kernel(X, dictionary, sparsity, **_run_kwargs):
    X = np.ascontiguousarray(np.asarray(X, dtype=np.float32))
    D = np.ascontiguousarray(np.asarray(dictionary, dtype=np.float32))
    S = int(np.asarray(sparsity))
    batch, feat = X.shape
    assert D.shape[0] == feat
    b_sh = batch // NCORES

    nc = build_program(S, b_sh=b_sh, feat=feat, atoms=D.shape[1])
    in_maps = [
        {"X": X[i * b_sh:(i + 1) * b_sh], "dictionary": D} for i in range(NCORES)
    ]
    res = run_bass_kernel_spmd(nc, in_maps, list(range(NCORES)), **_run_kwargs)
    out = np.concatenate([r["recon"] for r in res.results], axis=0)
    if getattr(res, "exec_time_ns", None) is not None:
        kernel.last_exec_time_ns = res.exec_time_ns
    kernel.last_results = res
    kernel.last_nc = nc
    kernel.last_in_maps = in_maps
    return out


kernel.last_exec_time_ns = None
kernel.last_results = None

